# revision 1
# baseline (speedup 1.0000x reference)
"""Trainium2 Bass kernel for a 3-layer GraphConv GNN (N=100k, E=1.6M, F=128).

Strategy (8 NeuronCores):
- Nodes sharded by dst across cores (12500/core, padded to 12544 = 98 blocks
  of 128). Edges partitioned by dst owner so aggregation is core-local.
- Aggregation: gather source rows per 128-edge chunk (dma_gather, int16
  indices bucketed into <=25088-row ranges of the table) and scatter-add via
  one-hot selection matmuls into PSUM (S[e,d] = (dst_local==d) * norm_dst).
- Edges are packed contiguously per (group, bucket) segment with per-cell
  capacity = max over cores (SPMD-static layout); chunks that straddle a
  block boundary are consumed twice with complementary one-hot masks. This
  cuts gather descriptors ~14% vs per-cell 128-chunk rounding.
- Feature-major pipeline: psum_agg[f,d] -> W matmul -> relu+bias -> transpose
  -> *norm_src -> per-core table slice; AllGather slices into the full
  node-major table for the next layer's gathers.
- Final: fused [fc_W|attn_W] matmul, exp-based sigmoid gate, bias, softmax.
  All activations (Relu/Copy/Exp) live in one act table set, so the act
  table is loaded once (sigmoid would force per-block table reloads).
"""
import os
import sys

sys.path.insert(0, "/opt/trn_rl_repo")

import numpy as np
import ml_dtypes

N = 100000
E = 1600000
F = 128
NCLS = 8
NCORES = 8
NPC = 12500          # nodes per core
PADN = 12544         # padded nodes per core (98 * 128)
NB = 98              # dst blocks per core
TBL = PADN * NCORES  # table rows in AllGather layout (100352)
NBKT = 4
QW = TBL // NBKT     # bucket width 25088 (< 32768 so int16 local idx works)
G = 7                # blocks per group
NGRP = NB // G       # 14 groups

USE_BF16 = os.environ.get("GNN_F32", "0") != "1"
LAYERS = int(os.environ.get("GNN_LAYERS", "3"))
SKIP_AG = os.environ.get("GNN_SKIP_AG", "0") == "1"

_CACHE = {}


def _assign_nodes(src, dst):
    """Balanced node -> (core, slot) assignment.

    An edge's gather bucket is its src's table-row quarter, i.e. the core
    PAIR holding the src (QW = 2*PADN). Keeping each node inside its source
    cohort (v // 25000 -> cores {2q, 2q+1}) fixes every edge's bucket up
    front, so blocks can then be formed greedily to equalize per-bucket
    in-edge counts. That makes the static cell capacities (max over cores)
    track the mean, cutting gather padding from ~7% to ~1.5%.
    """
    COH = N // NBKT            # 25000 nodes per source cohort
    BPC = 2 * NB               # 196 blocks per cohort
    NBLK = NCORES * NB         # 784

    ebkt = src // COH
    nodecnt = np.zeros((N, NBKT), dtype=np.int32)
    np.add.at(nodecnt, (dst, ebkt), 1)
    tot = nodecnt.sum(axis=1)

    block_of = np.empty(N, np.int32)
    blkcnt = np.zeros((NBLK, NBKT), np.float64)
    blknodes = [[] for _ in range(NBLK)]
    for q in range(NBKT):
        nodes = np.arange(q * COH, (q + 1) * COH)
        nodes = nodes[np.argsort(-tot[nodes], kind="stable")]
        base = q * BPC
        cnt = blkcnt[base:base + BPC]
        fl = np.zeros(BPC, np.int32)
        for v in nodes:
            w = nodecnt[v]
            sc = ((cnt + w) ** 2).sum(axis=1)
            sc[fl >= 128] = np.inf
            j = int(np.argmin(sc))
            block_of[v] = base + j
            blknodes[base + j].append(v)
            cnt[j] += w
            fl[j] += 1

    # round (= shared block index) assignment: 2 blocks per cohort per round
    order = np.argsort(-blkcnt.sum(axis=1), kind="stable")
    rmax = np.zeros((NB, NBKT), np.float64)
    percoh = np.zeros((NB, NBKT), np.int32)
    round_of = np.empty(NBLK, np.int32)
    for j in order:
        q = j // BPC
        inc = (np.maximum(rmax, blkcnt[j]) - rmax).sum(axis=1)
        inc[percoh[:, q] >= 2] = np.inf
        r = int(np.argmin(inc))
        round_of[j] = r
        rmax[r] = np.maximum(rmax[r], blkcnt[j])
        percoh[r, q] += 1

    core_of_block = np.empty(NBLK, np.int32)
    seen = {}
    for j in range(NBLK):
        key = (int(round_of[j]), j // BPC)
        core_of_block[j] = 2 * (j // BPC) + seen.get(key, 0)
        seen[key] = seen.get(key, 0) + 1

    core_of = np.empty(N, np.int32)
    slot_of = np.empty(N, np.int32)
    for j in range(NBLK):
        vs = blknodes[j]
        c = core_of_block[j]
        s0 = round_of[j] * 128
        for p, v in enumerate(vs):
            core_of[v] = c
            slot_of[v] = s0 + p
    return core_of, slot_of


def _host_schedule(src, dst):
    """Partition/sort edges; emit per-core gather/scatter schedule arrays.

    Static layout (same for all cores, SPMD): per (group g, bucket k) the
    cells (b, k) for the 7 blocks b of the group are packed contiguously,
    each with capacity cap[b,k] = max over cores of that cell's edge count.
    The segment is rounded up to whole 128-edge chunks. Chunk -> block
    "uses" may straddle block boundaries; each use gets its own one-hot
    column (dstl/enorm) with foreign/pad slots masked to 999/0.
    """
    src = np.asarray(src, dtype=np.int64)
    dst = np.asarray(dst, dtype=np.int64)

    deg_out = np.bincount(src, minlength=N).astype(np.float32)
    deg_in = np.bincount(dst, minlength=N).astype(np.float32)
    norm_src = np.where(deg_out > 0, 1.0 / np.sqrt(np.maximum(deg_out, 1.0)), 0.0).astype(np.float32)
    norm_dst = np.where(deg_in > 0, 1.0 / np.sqrt(np.maximum(deg_in, 1.0)), 0.0).astype(np.float32)

    core_of, slot_of = _assign_nodes(src, dst)
    rsrc_map = core_of.astype(np.int64) * PADN + slot_of
    rsrc = rsrc_map[src]
    owner = core_of[dst]

    per_core = []
    cnt_all = np.zeros((NCORES, NB, NBKT), dtype=np.int64)
    for c in range(NCORES):
        sel = owner == c
        es = rsrc[sel]
        ed = slot_of[dst[sel]].astype(np.int64)
        nd = norm_dst[dst[sel]]
        blk = ed >> 7
        dloc = (ed & 127).astype(np.float32)
        bkt = es // QW
        key = blk * NBKT + bkt
        order = np.argsort(key, kind="stable")
        es, dloc, nd, key = es[order], dloc[order], nd[order], key[order]
        cnt = np.bincount(key, minlength=NB * NBKT).reshape(NB, NBKT)
        cnt_all[c] = cnt
        per_core.append((es, dloc, nd, cnt))

    cap = cnt_all.max(axis=0)  # [NB, NBKT] static cell capacities

    # static segment layout: group g -> bucket k -> blocks packed contiguously
    segstart = np.zeros((NGRP, NBKT), dtype=np.int64)  # global chunk index
    segchunks = np.zeros((NGRP, NBKT), dtype=np.int64)
    segidx = np.zeros((NGRP, NBKT), dtype=np.int64)    # exact gather idx count
    cellpos = np.zeros((NB, NBKT), dtype=np.int64)     # slot offset in segment
    q = 0
    for g in range(NGRP):
        for k in range(NBKT):
            segstart[g, k] = q
            pos = 0
            for b in range(g * G, (g + 1) * G):
                cellpos[b, k] = pos
                pos += cap[b, k]
            nch = -(-pos // 128)
            segchunks[g, k] = nch
            segidx[g, k] = -(-pos // 16) * 16  # trim the pure segment tail
            q += nch
    T = int(q)

    # chunk -> block uses (static): processing order g -> b -> k -> chunk
    uses_by_block = [[] for _ in range(NB)]  # (k, chunk_global, use_col)
    ucol = 0
    for g in range(NGRP):
        for b in range(g * G, (g + 1) * G):
            for k in range(NBKT):
                if cap[b, k] == 0:
                    continue
                s0 = cellpos[b, k]
                s1 = s0 + cap[b, k]
                for cc in range(s0 // 128, -(-s1 // 128)):
                    uses_by_block[b].append((k, int(segstart[g, k] + cc), ucol))
                    ucol += 1
    U = ucol

    cores = []
    for c in range(NCORES):
        es, dloc, nd, cnt = per_core[c]
        off = np.zeros(NB * NBKT + 1, dtype=np.int64)
        np.cumsum(cnt.reshape(-1), out=off[1:])
        idx_flat = np.zeros(T * 128, dtype=np.int16)
        dstl_use = np.full((128, U), 999.0, dtype=np.float32)
        enorm_use = np.zeros((128, U), dtype=np.float32)
        for g in range(NGRP):
            for b in range(g * G, (g + 1) * G):
                for k in range(NBKT):
                    n = int(cnt[b, k])
                    capn = int(cap[b, k])
                    if capn == 0:
                        continue
                    s0 = int(off[b * NBKT + k])
                    slot0 = int(segstart[g, k]) * 128 + int(cellpos[b, k])
                    if n:
                        idx_flat[slot0:slot0 + n] = (es[s0:s0 + n] - k * QW).astype(np.int16)
        # per-use one-hot columns
        for b in range(NB):
            g = b // G
            for (k, cq, u) in uses_by_block[b]:
                n = int(cnt[b, k])
                if n == 0:
                    continue
                s0 = int(off[b * NBKT + k])
                slot0 = int(segstart[g, k]) * 128 + int(cellpos[b, k])  # cell start
                ch0 = cq * 128                                          # chunk start
                lo = max(slot0, ch0)
                hi = min(slot0 + n, ch0 + 128)
                if lo >= hi:
                    continue
                r0, r1 = lo - ch0, hi - ch0
                i0, i1 = lo - slot0, hi - slot0
                dstl_use[r0:r1, u] = dloc[s0 + i0:s0 + i1]
                enorm_use[r0:r1, u] = nd[s0 + i0:s0 + i1]
        # wrap idx per (g,k) gather segment: [16, n/16], idx i at [i%16, i//16]
        idx_w = np.zeros((16, T * 8), dtype=np.int16)
        for g in range(NGRP):
            for k in range(NBKT):
                q0 = int(segstart[g, k])
                nch = int(segchunks[g, k])
                if nch == 0:
                    continue
                seg = idx_flat[q0 * 128:(q0 + nch) * 128]
                idx_w[:, q0 * 8:(q0 + nch) * 8] = seg.reshape(-1, 16).T
        cores.append({
            "idx16": np.tile(idx_w, (8, 1)),
            "dstl": dstl_use,
            "enorm": enorm_use,
        })
    sched = {
        "T": T,
        "U": U,
        "segstart": segstart,
        "segchunks": segchunks,
        "segidx": segidx,
        "uses_by_block": uses_by_block,
    }
    return sched, cores, norm_src, norm_dst, core_of, slot_of


def _build_nc(sched, attn_b_val):
    import concourse.mybir as mybir
    import concourse.bacc as bacc
    import concourse.tile as tile
    from concourse.masks import make_identity

    DT = mybir.dt.bfloat16 if USE_BF16 else mybir.dt.float32
    f32 = mybir.dt.float32

    T = sched["T"]
    U = sched["U"]
    segstart = sched["segstart"]
    segchunks = sched["segchunks"]
    segidx = sched["segidx"]
    uses_by_block = sched["uses_by_block"]
    maxnch = int(segchunks.max())

    nc = bacc.Bacc("TRN2", target_bir_lowering=False, debug=False, num_devices=NCORES)
    t1_d = nc.dram_tensor("t1", [TBL, F], DT, kind="ExternalInput")
    idx_d = nc.dram_tensor("idx16", [128, T * 8], mybir.dt.int16, kind="ExternalInput")
    dstl_d = nc.dram_tensor("dstl", [128, U], f32, kind="ExternalInput")
    enorm_d = nc.dram_tensor("enorm", [128, U], f32, kind="ExternalInput")
    ns_d = nc.dram_tensor("nsb", [128, NB], f32, kind="ExternalInput")
    w_d = [nc.dram_tensor(f"w{i}", [F, F], DT, kind="ExternalInput") for i in (1, 2, 3)]
    b_d = [nc.dram_tensor(f"b{i}", [F, 1], f32, kind="ExternalInput") for i in (1, 2, 3)]
    fca_d = nc.dram_tensor("fca", [F, NCLS + 1], DT, kind="ExternalInput")
    fcb_d = nc.dram_tensor("fcb", [128, NCLS], f32, kind="ExternalInput")
    probs_d = nc.dram_tensor("probs", [PADN, NCLS], f32, kind="ExternalOutput")

    with tile.TileContext(nc) as tc:
        with tc.tile_pool(name="const", bufs=1) as cpool, \
             tc.tile_pool(name="msgp", bufs=12 if USE_BF16 else 6) as msgp, \
             tc.tile_pool(name="sp", bufs=24) as spool, \
             tc.tile_pool(name="wk", bufs=6) as wk, \
             tc.tile_pool(name="hp", bufs=10) as hpool, \
             tc.tile_pool(name="pagg", bufs=2, space="PSUM") as pagg, \
             tc.tile_pool(name="ph", bufs=2, space="PSUM") as ph, \
             tc.tile_pool(name="pt", bufs=2, space="PSUM") as pt, \
             tc.tile_pool(name="pm", bufs=2, space="PSUM") as pm, \
             tc.tile_pool(name="dram", bufs=1, space="DRAM") as dram:

            # constants
            iota_i = cpool.tile([128, 128], mybir.dt.int32)
            nc.gpsimd.iota(iota_i[:], pattern=[[1, 128]], base=0, channel_multiplier=0)
            iota_dt = cpool.tile([128, 128], DT)
            nc.vector.tensor_copy(out=iota_dt[:], in_=iota_i[:])
            ident = cpool.tile([128, 128], DT)
            make_identity(nc, ident[:])

            idx_t = cpool.tile([128, T * 8], mybir.dt.int16)
            nc.sync.dma_start(out=idx_t[:], in_=idx_d.ap())
            dstl_t = cpool.tile([128, U], f32)
            nc.sync.dma_start(out=dstl_t[:], in_=dstl_d.ap())
            enorm_t = cpool.tile([128, U], f32)
            nc.sync.dma_start(out=enorm_t[:], in_=enorm_d.ap())
            ns_t = cpool.tile([128, NB], f32)
            nc.sync.dma_start(out=ns_t[:], in_=ns_d.ap())
            w_t = []
            b_t = []
            for i in range(3):
                wt = cpool.tile([F, F], DT, tag=f"w{i}")
                nc.sync.dma_start(out=wt[:], in_=w_d[i].ap())
                w_t.append(wt)
                bt = cpool.tile([F, 1], f32, tag=f"b{i}")
                nc.sync.dma_start(out=bt[:], in_=b_d[i].ap())
                b_t.append(bt)
            fca_t = cpool.tile([F, NCLS + 1], DT)
            nc.sync.dma_start(out=fca_t[:], in_=fca_d.ap())
            fcb_t = cpool.tile([128, NCLS], f32)
            nc.sync.dma_start(out=fcb_t[:], in_=fcb_d.ap())

            # One-time memset of the message pool slots: gathers with a
            # trimmed num_idxs leave the tail of the last chunk unwritten,
            # and the masked one-hot (0 x garbage) must not see NaNs from
            # uninitialized SBUF. After the first rotation, stale data is
            # always a previous finite gather.
            for _ in range(12 if USE_BF16 else 6):
                mz = msgp.tile([128, maxnch, F], DT, tag="msg")
                nc.vector.memset(mz[:], 0)

            # inter-layer tables
            tables = [t1_d.ap()]
            ccins = []
            for l in (2, 3):
                tbl = dram.tile([TBL, F], DT, tag=f"tbl{l}", addr_space="Shared")
                cci = dram.tile([PADN, F], DT, tag=f"cci{l}")
                tables.append(tbl[:])
                ccins.append(cci)

            for l in range(LAYERS):
                table_ap = tables[l]
                for g in range(NGRP):
                    msgs = {}
                    for k in range(NBKT):
                        nch = int(segchunks[g, k])
                        if nch == 0:
                            continue
                        q0 = int(segstart[g, k])
                        ni = int(segidx[g, k])
                        m = msgp.tile([128, nch, F], DT, tag="msg")
                        nc.gpsimd.dma_gather(
                            m[:], table_ap[k * QW:TBL, :],
                            idx_t[:, q0 * 8:q0 * 8 + ni // 16],
                            ni, ni, F, single_packet=False)
                        msgs[k] = (m, q0)
                    hs = {}
                    for b in range(g * G, (g + 1) * G):
                        uses = uses_by_block[b]
                        nuse = len(uses)
                        ps = pagg.tile([128, 128], f32, tag="pagg")
                        # In the drain phase (last groups of the last layer,
                        # all gathers already issued) Pool is idle; offload a
                        # quarter of the one-hot builds to shorten the
                        # DVE-bound tail.
                        drain = (l == LAYERS - 1) and (g >= NGRP - 4)
                        for i, (k, cq, u) in enumerate(uses):
                            m, q0 = msgs[k]
                            s_t = spool.tile([128, 128], DT, tag="s")
                            eng = nc.gpsimd if (drain and i % 4 == 3) else nc.vector
                            eng.tensor_scalar(
                                out=s_t[:], in0=iota_dt[:],
                                scalar1=dstl_t[:, u:u + 1],
                                scalar2=enorm_t[:, u:u + 1],
                                op0=mybir.AluOpType.is_equal,
                                op1=mybir.AluOpType.mult)
                            nc.tensor.matmul(
                                out=ps[:], lhsT=m[:, cq - q0, :], rhs=s_t[:],
                                start=(i == 0), stop=(i == nuse - 1))
                        aggT = wk.tile([128, 128], DT, tag="aggT")
                        nc.scalar.activation(aggT[:], ps[:],
                                             mybir.ActivationFunctionType.Copy,
                                             bias=0.0, scale=1.0)
                        psh = ph.tile([128, 128], f32, tag="ph")
                        nc.tensor.matmul(out=psh[:], lhsT=w_t[l][:], rhs=aggT[:],
                                         start=True, stop=True)
                        # Relu/Exp/Copy all live in the exp_and_others act
                        # table set (unlike Sigmoid), so one table load serves
                        # the whole kernel.
                        h_sb = hpool.tile([128, 128], DT, tag="h")
                        nc.scalar.activation(h_sb[:], psh[:],
                                             mybir.ActivationFunctionType.Relu,
                                             bias=b_t[l][:, :1], scale=1.0)
                        if l < LAYERS - 1:
                            pst = pt.tile([128, 128], DT, tag="pt")
                            nc.tensor.transpose(out=pst[:], in_=h_sb[:], identity=ident[:])
                            xt = wk.tile([128, 128], DT, tag="xt")
                            nc.scalar.activation(xt[:], pst[:],
                                                 mybir.ActivationFunctionType.Copy,
                                                 scale=ns_t[:, b:b + 1])
                            nc.sync.dma_start(
                                out=ccins[l][b * 128:(b + 1) * 128, :], in_=xt[:])
                        else:
                            hs[b] = h_sb
                    if l == LAYERS - 1:
                        # Second pass: classifier head + softmax. Deferring it
                        # keeps the PE queue free of ops that wait on the long
                        # Activation/DVE tails (head-of-line stalls).
                        for b in range(g * G, (g + 1) * G):
                            h_sb = hs[b]
                            pla = pm.tile([128, NCLS + 1], f32, tag="pla")
                            nc.tensor.matmul(out=pla[:], lhsT=h_sb[:], rhs=fca_t[:],
                                             start=True, stop=True)
                            # sigmoid(x + ab) = 1 / (1 + exp(-x - ab))
                            attn_e = wk.tile([128, 1], f32, tag="attn_e")
                            nc.scalar.activation(attn_e[:], pla[:, NCLS:NCLS + 1],
                                                 mybir.ActivationFunctionType.Exp,
                                                 bias=-float(attn_b_val), scale=-1.0)
                            attn_d = wk.tile([128, 1], f32, tag="attn_d")
                            nc.scalar.activation(attn_d[:], attn_e[:],
                                                 mybir.ActivationFunctionType.Copy,
                                                 bias=1.0, scale=1.0)
                            attn = wk.tile([128, 1], f32, tag="attn")
                            nc.vector.reciprocal(attn[:, :1], attn_d[:, :1])
                            logits = wk.tile([128, NCLS], f32, tag="logits")
                            nc.scalar.activation(logits[:], pla[:, :NCLS],
                                                 mybir.ActivationFunctionType.Copy,
                                                 scale=attn[:, :1])
                            nc.vector.tensor_tensor(
                                out=logits[:], in0=logits[:], in1=fcb_t[:],
                                op=mybir.AluOpType.add)
                            mx = wk.tile([128, 1], f32, tag="mx")
                            nc.vector.tensor_reduce(
                                out=mx[:], in_=logits[:], axis=mybir.AxisListType.X,
                                op=mybir.AluOpType.max)
                            sh = wk.tile([128, NCLS], f32, tag="sh")
                            nc.vector.tensor_scalar(
                                out=sh[:], in0=logits[:], scalar1=mx[:, :1],
                                scalar2=None, op0=mybir.AluOpType.subtract)
                            ex = wk.tile([128, NCLS], f32, tag="ex")
                            ssum = wk.tile([128, 1], f32, tag="ssum")
                            nc.scalar.activation(ex[:], sh[:],
                                                 mybir.ActivationFunctionType.Exp,
                                                 accum_out=ssum[:, :1])
                            rinv = wk.tile([128, 1], f32, tag="rinv")
                            nc.vector.reciprocal(rinv[:, :1], ssum[:, :1])
                            pr = wk.tile([128, NCLS], f32, tag="pr")
                            nc.scalar.activation(pr[:], ex[:],
                                                 mybir.ActivationFunctionType.Copy,
                                                 scale=rinv[:, :1])
                            nc.sync.dma_start(
                                out=probs_d.ap()[b * 128:(b + 1) * 128, :], in_=pr[:])
                if l < LAYERS - 1 and not SKIP_AG:
                    nc.gpsimd.collective_compute(
                        "AllGather", mybir.AluOpType.bypass,
                        replica_groups=[list(range(NCORES))],
                        ins=[ccins[l].opt()], outs=[tables[l + 1].tensor.ap()])
    nc.compile()
    return nc


def _prepare(inputs):
    src = inputs["src"]
    dst = inputs["dst"]
    sched, cores, norm_src, norm_dst, core_of, slot_of = _host_schedule(src, dst)

    np_dt = ml_dtypes.bfloat16 if USE_BF16 else np.float32

    feats = np.asarray(inputs["features"], dtype=np.float32)
    xt1 = feats * norm_src[:, None]
    t1 = np.zeros((TBL, F), dtype=np_dt)
    t1[core_of.astype(np.int64) * PADN + slot_of] = xt1.astype(np_dt)

    fca = np.concatenate([np.asarray(inputs["fc_W"], np.float32),
                          np.asarray(inputs["attn_W"], np.float32)], axis=1).astype(np_dt)
    fcb = np.tile(np.asarray(inputs["fc_b"], np.float32)[None, :], (128, 1))

    in_maps = []
    for c in range(NCORES):
        sel = core_of == c
        vals = np.zeros(PADN, dtype=np.float32)
        vals[slot_of[sel]] = norm_src[sel]
        ns_col = np.ascontiguousarray(vals.reshape(NB, 128).T)
        m = {
            "t1": t1,
            "idx16": cores[c]["idx16"],
            "dstl": cores[c]["dstl"],
            "enorm": cores[c]["enorm"],
            "nsb": ns_col,
            "fca": fca,
            "fcb": fcb.astype(np.float32),
        }
        for i, wn in enumerate(("W1", "W2", "W3")):
            m[f"w{i + 1}"] = np.asarray(inputs[wn], np.float32).astype(np_dt)
        for i, bn in enumerate(("b1", "b2", "b3")):
            m[f"b{i + 1}"] = np.asarray(inputs[bn], np.float32).reshape(F, 1)
        in_maps.append(m)

    attn_b_val = float(np.asarray(inputs["attn_b"]).reshape(-1)[0])
    return (sched, attn_b_val), in_maps, (core_of, slot_of)


def run(inputs, trace=False):
    from concourse.bass_utils import run_bass_kernel_spmd

    (sched, attn_b_val), in_maps, (core_of, slot_of) = _prepare(inputs)
    ck = ("nc", sched["T"], sched["U"], sched["segchunks"].tobytes(),
          sched["segidx"].tobytes(), USE_BF16, attn_b_val, LAYERS, SKIP_AG)
    if ck not in _CACHE:
        _CACHE[ck] = _build_nc(sched, attn_b_val)
    nc = _CACHE[ck]
    try:
        res = run_bass_kernel_spmd(nc, in_maps, core_ids=list(range(NCORES)), trace=trace)
    except ModuleNotFoundError:
        res = run_bass_kernel_spmd(nc, in_maps, core_ids=list(range(NCORES)), trace=False)
    out = np.empty((N, NCLS), dtype=np.float32)
    for c in range(NCORES):
        sel = core_of == c
        out[sel] = res.results[c]["probs"][slot_of[sel]]
    return out, res


def kernel(**inputs):
    return run(inputs)[0]

